# revision 23
# baseline (speedup 1.0000x reference)
"""Trainium2 Bass kernel for nn_CASAtt_MultiHead_v1 (CAS attention block).

Reference computation (per sample):
    qkv = 1x1 conv (qkv_w) -> q, k, v                        [512, 56, 56] each
    q <- SE(dwconv3x3(q, sq_w, sq_b))   (per-head squeeze-excite)
    k <- SE(dwconv3x3(k, sk_w, sk_b))
    out = proj(dwconv3x3(q + k, dwc_w, dwc_b) * v) + proj_b + x

Distribution: data-parallel over batch, 2 samples per NeuronCore x 8 cores.

v3 design (on top of the fp8 DoubleRow v2 scheme):
  - SE border-stats ride the qkv GEMM: 8 columns of x strip/corner sums
    are appended to the x8 moving tile; the t=6 GEMM matmul runs FD=456
    and its last 8 psum columns are W*(x strips) = the q~/k~ strip sums
    (1x1 conv commutes with spatial sums), so the per-(br,oc) DVE strip
    reduces + 8-op serial combine collapse to a dot (TT + reduce).
  - SE is batched: one [128,128] se1 matmul covers 4 heads (block cols),
    one batched relu, 8 tiny se2 matmuls off hv quadrants.
  - conv1 stationary scaling (x sigmoid s) moved DVE -> ACT (4 ops/oc,
    tensor scale), emitted during the v GEMMs so conv1 never stalls.
  - DMA: host-side repack so every load is >=1.4KB-contiguous per
    partition (dg1/dg2/cpack were 256B descriptors), x8 split by kc
    plane (3136B descs), wq/wk on the ACT hwdge queue so the first
    GEMM starts ~3us in, next sample's x8 prefetched before this
    sample's out stores enter the sync queue. Residual xf is bf16.
  - phase 2 interleaves conv2(t) with proj(t-1) (one-tile lag) so proj
    never waits on the o2 drain chain; pad-zero memsets run on GPSIMD.
"""

import numpy as np

DIM = 512
NH = 4
HD = 128
HD4 = 32
B, H_FULL, W = 16, 56, 56
N_CORES = 8

H = 56
WP = 58
HP = 58
PADN = HP * WP          # 3364
HEAD = 16               # head slop so (row0-1, col-1) reads stay in-bounds
BUFN = HEAD + PADN + 12  # 3392, multiple of 16
TH = 8
NT = H // TH            # 7
TN = TH * W             # 448
FDC = TH * WP           # 464  (conv matmul moving/psum free size)
HW = H * W              # 3136
NST = 8                 # appended stat columns on the x8 tile
HWS = HW + NST          # 3144

SCALE_W = 32.0          # host-side scale on qkv/conv1/conv2/proj weights
CSCALE = 16.0           # extra scale kept on c2t/o2 for fp8 range


def default_cfg():
    return dict(
        b_local=B // N_CORES,
        conv2_midpair=True,
    )


def build_nc(cfg):
    import concourse.bass as bass
    import concourse.mybir as mybir
    import concourse.tile as tile
    from concourse import bacc
    from contextlib import ExitStack

    f32 = mybir.dt.float32
    bf16 = mybir.dt.bfloat16
    f8 = mybir.dt.float8e4
    DR = mybir.MatmulPerfMode.DoubleRow
    AF = mybir.ActivationFunctionType
    AL = mybir.AluOpType
    AX = mybir.AxisListType

    BL = cfg['b_local']

    nc = bacc.Bacc("TRN2", target_bir_lowering=False, debug=False,
                   enable_asserts=False, num_devices=N_CORES)

    # ---------------- DRAM I/O ----------------
    x8_d = nc.dram_tensor("x8", [BL, DIM, H, W], f8, kind="ExternalInput").ap()
    xf_d = nc.dram_tensor("xf", [BL, DIM, H, W], bf16,
                          kind="ExternalInput").ap()
    out_d = nc.dram_tensor("out", [BL, DIM, H, W], bf16,
                           kind="ExternalOutput").ap()
    wg_d = {n: nc.dram_tensor(n, [HD, NH, DIM], f8, kind="ExternalInput").ap()
            for n in ("wq", "wk", "wv", "wp")}
    # p-major packs: per-partition contiguous DMA descriptors
    dg1_d = nc.dram_tensor("dg1", [HD, NH, 9, 2 * HD], f8,
                           kind="ExternalInput").ap()
    dg2p_d = nc.dram_tensor("dg2p", [HD, NH, 4, 2 * HD], f8,
                            kind="ExternalInput").ap()
    dg2s_d = nc.dram_tensor("dg2s", [HD, NH, HD], f8,
                            kind="ExternalInput").ap()
    # packed per-(br,oc) consts, HD partitions:
    #   cols 0-8 pv, 9-40 unused(sew1), 41 seb2, 42 bias1, 43 dwcb (br=0)
    cpack_d = nc.dram_tensor("cpack", [HD, 2, NH, 44], f32,
                             kind="ExternalInput").ap()
    # se2 stationary block-stacked per head quadrant: [32f+j, br, o]
    sew2b_d = nc.dram_tensor("sew2b", [HD, 2, HD], f32,
                             kind="ExternalInput").ap()
    # batched se1 stationary [p, br, 32*oc+j] and relu bias [p, br]
    sew1b_d = nc.dram_tensor("sew1b", [HD, 2, NH * HD4], f32,
                             kind="ExternalInput").ap()
    sb1_d = nc.dram_tensor("sb1", [HD, 2], f32, kind="ExternalInput").ap()

    def pairify(ap2d, start, fd, pstride):
        """[128, fd] slice at `start` -> [128, 2, fd] with plane stride."""
        u = ap2d[:, start:start + fd].unsqueeze(1)
        a = u.ap
        a.pop(1)
        a.insert(1, (pstride, 2))
        return u

    with tile.TileContext(nc) as tc, ExitStack() as ctx:
        const = ctx.enter_context(tc.tile_pool(name="const", bufs=1))
        small = ctx.enter_context(tc.tile_pool(name="small", bufs=24))
        wres = ctx.enter_context(tc.tile_pool(name="wres", bufs=1))
        dg1s_p = ctx.enter_context(tc.tile_pool(name="dg1s", bufs=1))
        big = ctx.enter_context(tc.tile_pool(name="big", bufs=1))
        x8pool = ctx.enter_context(tc.tile_pool(name="x8p", bufs=2))
        xfpool = ctx.enter_context(tc.tile_pool(name="xfp", bufs=8))
        c2pool = ctx.enter_context(tc.tile_pool(name="c2p", bufs=3))
        stage = ctx.enter_context(tc.tile_pool(name="stage", bufs=6))
        statp = ctx.enter_context(tc.tile_pool(name="statp", bufs=12))
        mmp = ctx.enter_context(tc.tile_pool(name="mmp", bufs=7, space="PSUM"))
        sep = ctx.enter_context(tc.tile_pool(name="sep", bufs=1, space="PSUM"))

        # ---------- input DMA (emission order == queue order) ----------
        # wq/wk ride the ACT hwdge queue (idle at startup); x8 + the rest
        # ride the sync queue with x8 kc-plane chunks (3136B descriptors)
        # first so the first GEMM can start ~3us in.
        # first GEMM needs wq kc01 + x8 kc01 rows 0-13; split across the
        # two hwdge queues so transfers run concurrently, quartered so the
        # DMA engines interleave multiple in-flight transfers
        wg = {}
        HW4 = HW // 4
        t = wres.tile([HD, NH, DIM], f8, name="wq_sb")
        nc.sync.dma_start(t[:, 0:2, :], wg_d["wq"][:, 0:2, :])
        nc.scalar.dma_start(t[:, 2:4, :], wg_d["wq"][:, 2:4, :])
        wg["wq"] = t
        x8_first = x8pool.tile([HD, NH, HWS], f8, tag="x8", name="x8_b0")
        x8_src0 = x8_d[0].rearrange("(kc p) h w -> p kc (h w)", p=HD)
        HW8 = HW // 8
        for q in range(8):
            sl = slice(q * HW8, (q + 1) * HW8)
            nc.sync.dma_start(x8_first[:, 0:2, sl], x8_src0[:, 0:2, sl])
            nc.scalar.dma_start(x8_first[:, 2:4, sl], x8_src0[:, 2:4, sl])
        t = wres.tile([HD, NH, DIM], f8, name="wk_sb")
        nc.sync.dma_start(t[:, 0:2, :], wg_d["wk"][:, 0:2, :])
        nc.scalar.dma_start(t[:, 2:4, :], wg_d["wk"][:, 2:4, :])
        wg["wk"] = t
        cpack = wres.tile([HD, 2, NH, 44], f32, name="cpack_sb")
        nc.scalar.dma_start(cpack, cpack_d)
        sew1b = wres.tile([HD, 2, NH * HD4], f32, name="sew1b_sb")
        nc.scalar.dma_start(sew1b, sew1b_d)
        sew2b = wres.tile([HD, 2, HD], f32, name="sew2b_sb")
        nc.scalar.dma_start(sew2b, sew2b_d)
        sb1 = wres.tile([HD, 2], f32, name="sb1_sb")
        nc.scalar.dma_start(sb1, sb1_d)
        t = wres.tile([HD, NH, DIM], f8, name="wv_sb")
        nc.sync.dma_start(t, wg_d["wv"])
        wg["wv"] = t
        t = wres.tile([HD, NH, DIM], f8, name="wp_sb")
        nc.scalar.dma_start(t, wg_d["wp"])
        wg["wp"] = t
        dg1 = []
        for c in range(NH):
            t = wres.tile([HD, 9, 2 * HD], f8, name=f"dg1_{c}")
            nc.sync.dma_start(t, dg1_d[:, c])
            dg1.append(t)
        dg2p = []
        for c in range(NH):
            t = wres.tile([HD, 4, 2 * HD], f8, name=f"dg2p_{c}")
            nc.sync.dma_start(t, dg2p_d[:, c])
            dg2p.append(t)
        dg2s = []
        for c in range(NH):
            t = wres.tile([HD, HD], f8, name=f"dg2s_{c}")
            nc.sync.dma_start(t, dg2s_d[:, c])
            dg2s.append(t)
        pv = [[cpack[:, br, c, 0:9] for c in range(NH)] for br in range(2)]
        seb2 = [[cpack[:, br, c, 41:42] for c in range(NH)] for br in range(2)]
        bias1 = [[cpack[:, br, c, 42:43] for c in range(NH)] for br in range(2)]
        dwcb = [cpack[:, 0, c, 43:44] for c in range(NH)]

        # ---------- persistent SBUF ----------
        qk = [big.tile([HD, 2 * BUFN], f8, name=f"qk{c}") for c in range(NH)]
        m_t = [big.tile([HD, BUFN], f8, name=f"m{c}") for c in range(NH)]
        v_t = big.tile([HD, NH, HW], f8, name="v")
        o2_t = big.tile([HD, NH, HW], f8, name="o2")
        hvz = big.tile([HD, 2, NH], f32, name="hvz")
        nc.gpsimd.memset(hvz, 0.0)
        # dummy sigmoid: pulls the sigmoid act-table load (1.3us) into the
        # startup DMA wait instead of the first sample's SE chain
        warm = big.tile([HD, 1], f32, name="warm")
        nc.scalar.activation(warm, hvz[:, 0, 0:1], AF.Sigmoid, bias=0.0)

        # zero only the pad cells that valid conv outputs read (GPSIMD:
        # off the DVE critical path at startup)
        def zero_pads(plane_base, tbuf):
            for start, stride, count in (
                    (plane_base + HEAD, 1, 58),
                    (plane_base + HEAD + 57 * WP, 1, 58),
                    (plane_base + HEAD, WP, 58),
                    (plane_base + HEAD + 57, WP, 58)):
                u = tbuf[:, start:start + 1 + stride * (count - 1)]
                a = u.ap
                a.pop(1)
                a.insert(1, (stride, count))
                nc.gpsimd.memset(u, 0.0)

        for c in range(NH):
            zero_pads(0, qk[c])
            zero_pads(BUFN, qk[c])
            zero_pads(0, m_t[c])

        def interior(tbuf, plane_base, t):
            """[128, 8, 56] view of padded rows 1+8t..8+8t, cols 1..56."""
            base = plane_base + HEAD + (1 + TH * t) * WP + 1
            u = tbuf[:, base:base + TH * WP]
            return u.rearrange("p (r c) -> p r c", c=WP)[:, :, 0:W]

        def issue_x8(b):
            if b == 0:
                return x8_first
            x8 = x8pool.tile([HD, NH, HWS], f8, tag="x8", name=f"x8_b{b}")
            x8_src = x8_d[b].rearrange("(kc p) h w -> p kc (h w)", p=HD)
            for kc in range(0, NH, 2):
                nc.sync.dma_start(x8[:, kc:kc + 2, 0:HW],
                                  x8_src[:, kc:kc + 2, :])
            return x8

        def emit_sample(b, x8, x8_next):
            sfx = f"_b{b}"
            # ------- x strip/corner stats into the x8 tile's tail -------
            # col HW+j, j: 0 row0, 1 row55, 2 col0, 3 col55,
            #             4 c(0,0), 5 c(0,55), 6 c(55,0), 7 c(55,55)
            for j, (lo, step) in enumerate(
                    ((0, 1), (55 * W, 1), (0, W), (55, W))):
                src = x8[:, :, lo:lo + 1 + step * (W - 1)]
                a = src.ap
                a.pop(2)
                a.insert(2, (step, W))
                with nc.allow_low_precision(reason="fp8 strip sums feed "
                                            "small SE border corrections"):
                    nc.vector.tensor_reduce(x8[:, :, HW + j:HW + j + 1],
                                            src, AX.X, AL.add)
            for j, lo in ((4, 0), (6, 55 * W)):
                src = x8[:, :, lo:lo + 56]
                a = src.ap
                a.pop(2)
                a.insert(2, (55, 2))
                nc.vector.tensor_scalar(x8[:, :, HW + j:HW + j + 2],
                                        src, 1.0, None, AL.mult)

            xfs = []
            for ocp in range(NH):
                xf = xfpool.tile([HD, HW], bf16, tag="xf",
                                 name=f"xf{ocp}{sfx}")
                nc.sync.dma_start(
                    xf.rearrange("p (h w) -> p h w", w=W),
                    xf_d[b, ocp * HD:(ocp + 1) * HD])
                xfs.append(xf)

            # ---- qk GEMM (br-major); t=6 runs FD=456, its last 8 psum
            # columns are the W*(x strips) = q~/k~ border sums ----
            stats = [None] * NH
            corr4 = small.tile([HD, 2, NH], f32, tag="corr4",
                               name=f"corr4{sfx}")
            s_sb = [[None] * NH for _ in range(2)]

            def v_gemm(oc):
                for t in range(NT):
                    ps = mmp.tile([HD, TN], f32, tag="mm",
                                  name=f"v{oc}_{t}{sfx}")
                    for kp in range(2):
                        nc.tensor.matmul(
                            ps,
                            wg["wv"][:, 2 * kp:2 * kp + 2,
                                     oc * HD:(oc + 1) * HD],
                            x8[:, 2 * kp:2 * kp + 2, t * TN:(t + 1) * TN],
                            start=(kp == 0), stop=(kp == 1), perf_mode=DR)
                    nc.vector.tensor_scalar(
                        v_t[:, oc, t * TN:(t + 1) * TN], ps,
                        1.0 / SCALE_W, None, AL.mult)

            def se_branch(br):
                # corr for all 4 oc of this branch is ready shortly after
                # its last qk group; emitted right behind it so the chain
                # overlaps the next branch's / v's GEMMs on PE
                ps1 = sep.tile([HD, NH], f32, tag="se",
                               name=f"se1_{br}{sfx}")
                nc.tensor.matmul(ps1, sew1b[:, br, :], corr4[:, br, :],
                                 start=True, stop=True)
                for f in range(NH):
                    nc.scalar.activation(
                        hvz[f * HD4:(f + 1) * HD4, br, f:f + 1],
                        ps1[f * HD4:(f + 1) * HD4, f:f + 1],
                        AF.Relu, bias=sb1[f * HD4:(f + 1) * HD4,
                                          br:br + 1])
                ps2 = sep.tile([HD, NH], f32, tag="se",
                               name=f"se2_{br}{sfx}")
                nc.tensor.matmul(ps2, sew2b[:, br, :], hvz[:, br, :],
                                 start=True, stop=True)
                for oc in range(NH):
                    sv = small.tile([HD, 1], f32, tag="sv",
                                    name=f"s{br}_{oc}{sfx}")
                    nc.scalar.activation(sv, ps2[:, oc:oc + 1], AF.Sigmoid,
                                         bias=seb2[br][oc])
                    s_sb[br][oc] = sv

            for br in range(2):
                for oc in range(NH):
                    if br == 0:
                        stats[oc] = statp.tile([HD, 2, 9], f32, tag="stat",
                                               name=f"stat{oc}{sfx}")
                    wt = wg["wq"] if br == 0 else wg["wk"]
                    st = statp.tile([HD, NT], f32, tag="st",
                                    name=f"st{br}_{oc}{sfx}")
                    for t in range(NT):
                        fd = TN + NST if t == NT - 1 else TN
                        ps = mmp.tile([HD, fd], f32, tag="mm",
                                      name=f"g{br}_{oc}_{t}{sfx}")
                        for kp in range(2):
                            nc.tensor.matmul(
                                ps,
                                wt[:, 2 * kp:2 * kp + 2,
                                   oc * HD:(oc + 1) * HD],
                                x8[:, 2 * kp:2 * kp + 2,
                                   t * TN:t * TN + fd],
                                start=(kp == 0), stop=(kp == 1),
                                perf_mode=DR)
                        if t % 2 == 0 and t != NT - 1:
                            nc.scalar.activation(
                                interior(qk[oc], br * BUFN, t),
                                ps[:, 0:TN].rearrange(
                                    "p (r c) -> p r c", c=W),
                                AF.Identity, bias=0.0, scale=1.0 / SCALE_W,
                                accum_out=st[:, t:t + 1])
                        else:
                            nc.vector.tensor_scalar(
                                interior(qk[oc], br * BUFN, t),
                                ps[:, 0:TN].rearrange(
                                    "p (r c) -> p r c", c=W),
                                1.0 / SCALE_W, 0.0, AL.mult, AL.add,
                                accum_out=st[:, t:t + 1])
                        if t == NT - 1:
                            nc.vector.tensor_scalar(
                                stats[oc][:, br, 1:9], ps[:, TN:TN + NST],
                                1.0 / SCALE_W, None, AL.mult)
                    nc.vector.tensor_reduce(stats[oc][:, br, 0:1],
                                            st[:, 0:NT], AX.X, AL.add)
                    tmp9 = small.tile([HD, 9], f32, tag="tmp9", name=None)
                    nc.gpsimd.tensor_tensor(tmp9, stats[oc][:, br, :],
                                            pv[br][oc], AL.mult)
                    nc.vector.tensor_reduce(corr4[:, br, oc:oc + 1],
                                            tmp9, AX.X, AL.add)
                if br == 0:
                    se_branch(0)
                else:
                    v_gemm(0)
                    se_branch(1)
                    v_gemm(1)
                    v_gemm(2)

            # ------- v GEMM oc 3 (filler while dg1s scaling runs) -------
            v_gemm(3)

            # prefetch next sample's x8 ahead of this sample's out stores
            if x8_next is not None:
                x8n = issue_x8(b + 1)
            else:
                x8n = None

            # ------- scale conv1 stationaries (ACT) + bias_m -------
            dg1s = [None] * NH
            bias_m = [None] * NH
            for oc in range(NH):
                gt = dg1s_p.tile([HD, 9, 2 * HD], f8, tag=f"dg1s{oc}",
                                 name=f"dg1s{oc}{sfx}")
                nc.scalar.activation(gt[:, 0:3, :], dg1[oc][:, 0:3, :],
                                     AF.Identity, bias=0.0,
                                     scale=s_sb[0][oc])
                nc.vector.tensor_scalar(gt[:, 3:6, :], dg1[oc][:, 3:6, :],
                                        s_sb[1][oc], None, AL.mult)
                gc = gt[:, 6:9, :].rearrange("p j (two f) -> p j two f",
                                             two=2)
                dc = dg1[oc][:, 6:9, :].rearrange("p j (two f) -> p j two f",
                                                  two=2)
                nc.scalar.activation(gc[:, :, 0, :], dc[:, :, 0, :],
                                     AF.Identity, bias=0.0,
                                     scale=s_sb[0][oc])
                nc.vector.tensor_scalar(gc[:, :, 1, :], dc[:, :, 1, :],
                                        s_sb[1][oc], None, AL.mult)
                dg1s[oc] = gt
                bm = small.tile([HD, 1], f32, tag="bm", name=f"bm{oc}{sfx}")
                tmp = small.tile([HD, 1], f32, tag="bmt", name=None)
                nc.vector.tensor_scalar(tmp, bias1[0][oc], s_sb[0][oc],
                                        None, AL.mult)
                nc.vector.scalar_tensor_tensor(bm, bias1[1][oc], s_sb[1][oc],
                                               tmp, AL.mult, AL.add)
                bias_m[oc] = bm

            # ---------------- conv1 (fused q+k -> m) ----------------
            for oc in range(NH):
                qplane = qk[oc]
                for t in range(NT):
                    y0 = 1 + TH * t
                    ps = mmp.tile([HD, FDC], f32, tag="mm",
                                  name=f"c1_{oc}_{t}{sfx}")
                    mms = []
                    for i in range(9):
                        grp, dx = divmod(i, 3)
                        dx -= 1
                        if grp == 0:    # q vertical pairs
                            mv = pairify(qplane,
                                         HEAD + (y0 - 1) * WP + dx,
                                         FDC, 2 * WP)
                        elif grp == 1:  # k vertical pairs
                            mv = pairify(qplane,
                                         BUFN + HEAD + (y0 - 1) * WP + dx,
                                         FDC, 2 * WP)
                        else:           # cross q/k middle row
                            mv = pairify(qplane, HEAD + y0 * WP + dx,
                                         FDC, BUFN)
                        st = dg1s[oc][:, i, :]
                        st = st.rearrange("p (two f) -> p two f", two=2)
                        mms.append((st, mv))
                    for i, (st, mv) in enumerate(mms):
                        nc.tensor.matmul(ps, st, mv, start=(i == 0),
                                         stop=(i == 8), perf_mode=DR)
                    nc.scalar.activation(
                        interior(m_t[oc], 0, t),
                        ps.rearrange("p (r c) -> p r c", c=WP)[:, :, 1:1 + W],
                        AF.Identity, bias=bias_m[oc], scale=1.0 / SCALE_W)

            # ------- conv2(t) + proj(t-1), one-tile lag -------
            stgs = [None] * NH
            for t in range(NT + 1):
                if t < NT:
                    for oc in range(NH):
                        y0 = 1 + TH * t
                        ps = mmp.tile([HD, FDC], f32, tag="mm",
                                      name=f"c2_{oc}_{t}{sfx}")
                        for i in range(3):
                            dx = i - 1
                            st = dg2p[oc][:, i, :]
                            st = st.rearrange("p (two f) -> p two f", two=2)
                            mv = pairify(m_t[oc], HEAD + (y0 - 1) * WP + dx,
                                         FDC, 2 * WP)
                            nc.tensor.matmul(ps, st, mv, start=(i == 0),
                                             stop=False, perf_mode=DR)
                        st = dg2p[oc][:, 3, :]
                        st = st.rearrange("p (two f) -> p two f", two=2)
                        mv = pairify(m_t[oc], HEAD + y0 * WP - 1, FDC, 2)
                        nc.tensor.matmul(ps, st, mv, start=False, stop=False,
                                         perf_mode=DR)
                        nc.tensor.matmul(
                            ps, dg2s[oc],
                            m_t[oc][:, HEAD + y0 * WP:HEAD + y0 * WP + FDC],
                            start=False, stop=True)
                        c2t = c2pool.tile([HD, TN], f8, tag="c2t",
                                          name=f"c2t{oc}_{t}{sfx}")
                        nc.scalar.activation(
                            c2t.rearrange("p (r c) -> p r c", c=W),
                            ps.rearrange("p (r c) -> p r c",
                                         c=WP)[:, :, 1:1 + W],
                            AF.Identity, bias=dwcb[oc],
                            scale=CSCALE / SCALE_W)
                        eng = nc.vector if oc == 0 else nc.gpsimd
                        eng.tensor_tensor(
                            o2_t[:, oc, t * TN:(t + 1) * TN], c2t,
                            v_t[:, oc, t * TN:(t + 1) * TN], AL.mult)
                if t >= 1:
                    tp = t - 1
                    for ocp in range(NH):
                        xf = xfs[ocp]
                        ps = mmp.tile([HD, TN], f32, tag="mm",
                                      name=f"p{ocp}_{tp}{sfx}")
                        for kp in range(2):
                            nc.tensor.matmul(
                                ps,
                                wg["wp"][:, 2 * kp:2 * kp + 2,
                                         ocp * HD:(ocp + 1) * HD],
                                o2_t[:, 2 * kp:2 * kp + 2,
                                     tp * TN:(tp + 1) * TN],
                                start=(kp == 0), stop=(kp == 1),
                                perf_mode=DR)
                        if tp % 2 == 0:
                            stg = stage.tile([HD, 2, TN], bf16, tag="stg",
                                             name=f"stg{ocp}_{tp}{sfx}")
                            stgs[ocp] = stg
                        else:
                            stg = stgs[ocp]
                        nc.vector.scalar_tensor_tensor(
                            stg[:, tp % 2, :], ps, 1.0 / (SCALE_W * CSCALE),
                            xf[:, tp * TN:(tp + 1) * TN], AL.mult, AL.add)
                        if tp % 2 == 1 or tp == NT - 1:
                            t0o = tp - (tp % 2)
                            deng = nc.sync if ocp % 2 == 0 else nc.scalar
                            deng.dma_start(
                                out_d[b, ocp * HD:(ocp + 1) * HD,
                                      TH * t0o:TH * (tp + 1), :],
                                stg[:, 0:tp - t0o + 1, :].rearrange(
                                    "p u (r c) -> p (u r) c", c=W))
            return x8n

        x8 = issue_x8(0)
        for b in range(BL):
            x8 = emit_sample(b, x8, b + 1 < BL or None)

    nc.compile()
    return nc


# ---------------------------------------------------------------------------
# host-side weight prep
# ---------------------------------------------------------------------------

def prep_weights(inputs, cfg):
    import ml_dtypes
    f8 = ml_dtypes.float8_e4m3
    f32 = np.float32

    qkv_w = np.asarray(inputs['qkv_w'], f32)
    proj_w = np.asarray(inputs['proj_w'], f32)

    def gemm_tile(wmat):
        # [HD p, NH kc, DIM oc*128+col] = SCALE_W * W[ocol, kc*128+p]
        arr = (SCALE_W * wmat).reshape(DIM, NH, HD).transpose(2, 1, 0)
        return np.ascontiguousarray(arr).astype(f8)

    wq = gemm_tile(qkv_w[0:DIM])
    wk = gemm_tile(qkv_w[DIM:2 * DIM])
    wv = gemm_tile(qkv_w[2 * DIM:3 * DIM])
    wp = gemm_tile(proj_w)

    sq = np.asarray(inputs['sq_w'], f32).reshape(DIM, 3, 3) * SCALE_W
    sk = np.asarray(inputs['sk_w'], f32).reshape(DIM, 3, 3) * SCALE_W
    dw = np.asarray(inputs['dwc_w'], f32).reshape(DIM, 3, 3) * SCALE_W

    idx = np.arange(HD)

    def diag(vals):
        d = np.zeros((HD, HD), f32)
        d[idx, idx] = vals
        return d

    # conv1 pair stationaries: [NH, 9, HD, 2*HD]
    dg1 = np.zeros((NH, 9, HD, 2 * HD), f32)
    for c in range(NH):
        q = sq[c * HD:(c + 1) * HD]
        k = sk[c * HD:(c + 1) * HD]
        for i in range(3):      # q vertical pairs, dx = i-1
            dg1[c, i, :, 0:HD] = diag(q[:, 0, i])
            dg1[c, i, :, HD:] = diag(q[:, 2, i])
        for i in range(3):      # k vertical pairs
            dg1[c, 3 + i, :, 0:HD] = diag(k[:, 0, i])
            dg1[c, 3 + i, :, HD:] = diag(k[:, 2, i])
        for i in range(3):      # cross middle row
            dg1[c, 6 + i, :, 0:HD] = diag(q[:, 1, i])
            dg1[c, 6 + i, :, HD:] = diag(k[:, 1, i])

    dg2p = np.zeros((NH, 4, HD, 2 * HD), f32)
    dg2s = np.zeros((NH, HD, HD), f32)
    for c in range(NH):
        d = dw[c * HD:(c + 1) * HD]
        for i in range(3):      # vertical pairs
            dg2p[c, i, :, 0:HD] = diag(d[:, 0, i])
            dg2p[c, i, :, HD:] = diag(d[:, 2, i])
        dg2p[c, 3, :, 0:HD] = diag(d[:, 1, 0])   # (0,-1)
        dg2p[c, 3, :, HD:] = diag(d[:, 1, 2])    # (0,+1)
        dg2s[c] = diag(d[:, 1, 1])

    # pooled-correction vectors (natural scale, /npix folded in):
    # pooled = A*S + B1*R0 + B2*R1 + B3*C0 + B4*C1 + c00*q00 + ...
    npix = float(H * W)
    pvv = np.zeros((2, DIM, 9), f32)
    for br, wc in enumerate((sq / SCALE_W, sk / SCALE_W)):
        pvv[br, :, 0] = wc.sum(axis=(1, 2))
        pvv[br, :, 1] = -wc[:, 2, :].sum(axis=1)   # row 0 strip (dy=+1 taps)
        pvv[br, :, 2] = -wc[:, 0, :].sum(axis=1)   # row 55 strip (dy=-1)
        pvv[br, :, 3] = -wc[:, :, 2].sum(axis=1)   # col 0 strip (dx=+1)
        pvv[br, :, 4] = -wc[:, :, 0].sum(axis=1)   # col 55 strip (dx=-1)
        pvv[br, :, 5] = wc[:, 2, 2]                # q[0,0]
        pvv[br, :, 6] = wc[:, 2, 0]                # q[0,55]
        pvv[br, :, 7] = wc[:, 0, 2]                # q[55,0]
        pvv[br, :, 8] = wc[:, 0, 0]                # q[55,55]
    pvv /= npix

    sew1 = np.stack([
        np.asarray(inputs['cq_w1'], f32).transpose(0, 2, 1),
        np.asarray(inputs['ck_w1'], f32).transpose(0, 2, 1)])  # [2,NH,HD,HD4]
    seb1 = np.stack([np.asarray(inputs['cq_b1'], f32),
                     np.asarray(inputs['ck_b1'], f32)])        # [2,NH,HD4]
    sew2 = np.stack([
        np.asarray(inputs['cq_w2'], f32).transpose(0, 2, 1),
        np.asarray(inputs['ck_w2'], f32).transpose(0, 2, 1)])  # [2,NH,HD4,HD]
    seb2 = np.stack([np.asarray(inputs['cq_b2'], f32),
                     np.asarray(inputs['ck_b2'], f32)])        # [2,NH,HD]
    b1 = np.stack([np.asarray(inputs['sq_b'], f32),
                   np.asarray(inputs['sk_b'], f32)])           # [2,DIM]
    dwcb = CSCALE * np.asarray(inputs['dwc_b'], f32)           # [DIM]

    cpack = np.zeros((2, NH, HD, 44), f32)
    for br in range(2):
        for c in range(NH):
            sl = slice(c * HD, (c + 1) * HD)
            cpack[br, c, :, 0:9] = pvv[br, sl]
            cpack[br, c, :, 41] = seb2[br, c]
            cpack[br, c, :, 38] = np.exp(-seb2[br, c])
            cpack[br, c, :, 42] = b1[br, sl]
    for c in range(NH):
        cpack[0, c, :, 43] = dwcb[c * HD:(c + 1) * HD]
        cpack[0, c, :, 40] = (CSCALE / SCALE_W) * dw[c * HD:(c + 1) * HD, 1, 1]

    # batched se1 stationary [HD, 2, NH*HD4] and relu bias [HD, 2]
    sew1b = np.zeros((HD, 2, NH * HD4), f32)
    for br in range(2):
        sew1b[:, br, :] = np.concatenate(
            [sew1[br, f] for f in range(NH)], axis=1)
    sb1 = np.ascontiguousarray(seb1.reshape(2, HD).T)

    return dict(
        wq=wq, wk=wk, wv=wv, wp=wp,
        dg1=np.ascontiguousarray(
            dg1.transpose(2, 0, 1, 3)).astype(f8),
        dg2p=np.ascontiguousarray(
            dg2p.transpose(2, 0, 1, 3)).astype(f8),
        dg2s=np.ascontiguousarray(dg2s.transpose(1, 0, 2)).astype(f8),
        cpack=np.ascontiguousarray(cpack.transpose(2, 0, 1, 3)),
        sew2b=np.ascontiguousarray(
            sew2.reshape(2, NH * HD4, HD).transpose(1, 0, 2)),
        sew1b=sew1b, sb1=sb1,
    )


_CACHE = {}


def _get_compiled(cfg_key, cfg):
    if cfg_key not in _CACHE:
        _CACHE[cfg_key] = build_nc(cfg)
    return _CACHE[cfg_key]


def make_in_maps(inputs, cfg):
    import ml_dtypes
    w = prep_weights(inputs, cfg)
    x32 = np.asarray(inputs['x'], np.float32)
    x8 = np.clip(x32, -240, 240).astype(ml_dtypes.float8_e4m3)
    projb = np.asarray(inputs['proj_b'], np.float32)
    xf = (x32 + projb[None, :, None, None]).astype(ml_dtypes.bfloat16)
    BL = cfg['b_local']
    in_maps = []
    for core in range(N_CORES):
        mm = dict(w)
        mm['x8'] = np.ascontiguousarray(x8[core * BL:(core + 1) * BL])
        mm['xf'] = np.ascontiguousarray(xf[core * BL:(core + 1) * BL])
        in_maps.append(mm)
    return in_maps


def kernel(**inputs):
    from concourse import bass_utils
    cfg = default_cfg()
    nc = _get_compiled('main', cfg)
    in_maps = make_in_maps(inputs, cfg)
    res = bass_utils.run_bass_kernel_spmd(nc, in_maps,
                                          core_ids=list(range(N_CORES)))
    bl = cfg['b_local']
    out = np.empty((B, DIM, H_FULL, W), np.float32)
    for core in range(N_CORES):
        out[core * bl:(core + 1) * bl] = np.asarray(
            res.results[core]['out'], np.float32)
    return out


# revision 24
# speedup vs baseline: 1.0417x; 1.0417x over previous
"""Trainium2 Bass kernel for nn_CASAtt_MultiHead_v1 (CAS attention block).

Reference computation (per sample):
    qkv = 1x1 conv (qkv_w) -> q, k, v                        [512, 56, 56] each
    q <- SE(dwconv3x3(q, sq_w, sq_b))   (per-head squeeze-excite)
    k <- SE(dwconv3x3(k, sk_w, sk_b))
    out = proj(dwconv3x3(q + k, dwc_w, dwc_b) * v) + proj_b + x

Distribution: data-parallel over batch, 2 samples per NeuronCore x 8 cores.

v3 design (on top of the fp8 DoubleRow v2 scheme):
  - SE border-stats ride the qkv GEMM: 8 columns of x strip/corner sums
    are appended to the x8 moving tile; the t=6 GEMM matmul runs FD=456
    and its last 8 psum columns are W*(x strips) = the q~/k~ strip sums
    (1x1 conv commutes with spatial sums), so the per-(br,oc) DVE strip
    reduces + 8-op serial combine collapse to a dot (TT + reduce).
  - SE is batched: one [128,128] se1 matmul covers 4 heads (block cols),
    one batched relu, 8 tiny se2 matmuls off hv quadrants.
  - conv1 stationary scaling (x sigmoid s) moved DVE -> ACT (4 ops/oc,
    tensor scale), emitted during the v GEMMs so conv1 never stalls.
  - DMA: host-side repack so every load is >=1.4KB-contiguous per
    partition (dg1/dg2/cpack were 256B descriptors), x8 split by kc
    plane (3136B descs), wq/wk on the ACT hwdge queue so the first
    GEMM starts ~3us in, next sample's x8 prefetched before this
    sample's out stores enter the sync queue. Residual xf is bf16.
  - phase 2 interleaves conv2(t) with proj(t-1) (one-tile lag) so proj
    never waits on the o2 drain chain; pad-zero memsets run on GPSIMD.
"""

import numpy as np

DIM = 512
NH = 4
HD = 128
HD4 = 32
B, H_FULL, W = 16, 56, 56
N_CORES = 8

H = 56
WP = 58
HP = 58
PADN = HP * WP          # 3364
HEAD = 16               # head slop so (row0-1, col-1) reads stay in-bounds
BUFN = HEAD + PADN + 12  # 3392, multiple of 16
TH = 8
NT = H // TH            # 7
TN = TH * W             # 448
FDC = TH * WP           # 464  (conv matmul moving/psum free size)
HW = H * W              # 3136
NST = 8                 # appended stat columns on the x8 tile
HWS = HW + NST          # 3144

SCALE_W = 32.0          # host-side scale on qkv/conv1/conv2/proj weights
CSCALE = 16.0           # extra scale kept on c2t/o2 for fp8 range


def default_cfg():
    return dict(
        b_local=B // N_CORES,
        conv2_midpair=True,
    )


def build_nc(cfg):
    import concourse.bass as bass
    import concourse.mybir as mybir
    import concourse.tile as tile
    from concourse import bacc
    from contextlib import ExitStack

    f32 = mybir.dt.float32
    bf16 = mybir.dt.bfloat16
    f8 = mybir.dt.float8e4
    DR = mybir.MatmulPerfMode.DoubleRow
    AF = mybir.ActivationFunctionType
    AL = mybir.AluOpType
    AX = mybir.AxisListType

    BL = cfg['b_local']

    nc = bacc.Bacc("TRN2", target_bir_lowering=False, debug=False,
                   enable_asserts=False, num_devices=N_CORES)

    # ---------------- DRAM I/O ----------------
    x8_d = nc.dram_tensor("x8", [BL, DIM, H, W], f8, kind="ExternalInput").ap()
    xf_d = nc.dram_tensor("xf", [BL, DIM, H, W], bf16,
                          kind="ExternalInput").ap()
    out_d = nc.dram_tensor("out", [BL, DIM, H, W], bf16,
                           kind="ExternalOutput").ap()
    wg_d = {n: nc.dram_tensor(n, [HD, NH, DIM], f8, kind="ExternalInput").ap()
            for n in ("wq", "wk", "wv", "wp")}
    # p-major packs: per-partition contiguous DMA descriptors
    dg1_d = nc.dram_tensor("dg1", [HD, NH, 9, 2 * HD], f8,
                           kind="ExternalInput").ap()
    dg2p_d = nc.dram_tensor("dg2p", [HD, NH, 4, 2 * HD], f8,
                            kind="ExternalInput").ap()
    dg2s_d = nc.dram_tensor("dg2s", [HD, NH, HD], f8,
                            kind="ExternalInput").ap()
    # packed per-(br,oc) consts, HD partitions:
    #   cols 0-8 pv, 9-40 unused(sew1), 41 seb2, 42 bias1, 43 dwcb (br=0)
    cpack_d = nc.dram_tensor("cpack", [HD, 2, NH, 44], f32,
                             kind="ExternalInput").ap()
    # se2 stationary block-stacked per head quadrant: [32f+j, br, o]
    sew2b_d = nc.dram_tensor("sew2b", [HD, 2, HD], f32,
                             kind="ExternalInput").ap()
    # batched se1 stationary [p, br, 32*oc+j] and relu bias [p, br]
    sew1b_d = nc.dram_tensor("sew1b", [HD, 2, NH * HD4], f32,
                             kind="ExternalInput").ap()
    sb1_d = nc.dram_tensor("sb1", [HD, 2], f32, kind="ExternalInput").ap()

    def pairify(ap2d, start, fd, pstride):
        """[128, fd] slice at `start` -> [128, 2, fd] with plane stride."""
        u = ap2d[:, start:start + fd].unsqueeze(1)
        a = u.ap
        a.pop(1)
        a.insert(1, (pstride, 2))
        return u

    with tile.TileContext(nc) as tc, ExitStack() as ctx:
        const = ctx.enter_context(tc.tile_pool(name="const", bufs=1))
        small = ctx.enter_context(tc.tile_pool(name="small", bufs=24))
        wres = ctx.enter_context(tc.tile_pool(name="wres", bufs=1))
        dg1s_p = ctx.enter_context(tc.tile_pool(name="dg1s", bufs=1))
        big = ctx.enter_context(tc.tile_pool(name="big", bufs=1))
        x8pool = ctx.enter_context(tc.tile_pool(name="x8p", bufs=2))
        xfpool = ctx.enter_context(tc.tile_pool(name="xfp", bufs=8))
        c2pool = ctx.enter_context(tc.tile_pool(name="c2p", bufs=3))
        stage = ctx.enter_context(tc.tile_pool(name="stage", bufs=6))
        statp = ctx.enter_context(tc.tile_pool(name="statp", bufs=12))
        mmp = ctx.enter_context(tc.tile_pool(name="mmp", bufs=7, space="PSUM"))
        sep = ctx.enter_context(tc.tile_pool(name="sep", bufs=1, space="PSUM"))

        # ---------- input DMA (emission order == queue order) ----------
        # wq/wk ride the ACT hwdge queue (idle at startup); x8 + the rest
        # ride the sync queue with x8 kc-plane chunks (3136B descriptors)
        # first so the first GEMM can start ~3us in.
        # first GEMM needs wq kc01 + x8 kc01 rows 0-13; split across the
        # two hwdge queues so transfers run concurrently, quartered so the
        # DMA engines interleave multiple in-flight transfers
        wg = {}
        HW4 = HW // 4
        t = wres.tile([HD, NH, DIM], f8, name="wq_sb")
        nc.sync.dma_start(t[:, 0:2, :], wg_d["wq"][:, 0:2, :])
        nc.scalar.dma_start(t[:, 2:4, :], wg_d["wq"][:, 2:4, :])
        wg["wq"] = t
        x8_first = x8pool.tile([HD, NH, HWS], f8, tag="x8", name="x8_b0")
        x8_src0 = x8_d[0].rearrange("(kc p) h w -> p kc (h w)", p=HD)
        for q in range(4):
            sl = slice(q * HW4, (q + 1) * HW4)
            nc.sync.dma_start(x8_first[:, 0:2, sl], x8_src0[:, 0:2, sl])
            nc.scalar.dma_start(x8_first[:, 2:4, sl], x8_src0[:, 2:4, sl])
        t = wres.tile([HD, NH, DIM], f8, name="wk_sb")
        nc.sync.dma_start(t[:, 0:2, :], wg_d["wk"][:, 0:2, :])
        nc.scalar.dma_start(t[:, 2:4, :], wg_d["wk"][:, 2:4, :])
        wg["wk"] = t
        cpack = wres.tile([HD, 2, NH, 44], f32, name="cpack_sb")
        nc.scalar.dma_start(cpack, cpack_d)
        sew1b = wres.tile([HD, 2, NH * HD4], f32, name="sew1b_sb")
        nc.scalar.dma_start(sew1b, sew1b_d)
        sew2b = wres.tile([HD, 2, HD], f32, name="sew2b_sb")
        nc.scalar.dma_start(sew2b, sew2b_d)
        sb1 = wres.tile([HD, 2], f32, name="sb1_sb")
        nc.scalar.dma_start(sb1, sb1_d)
        t = wres.tile([HD, NH, DIM], f8, name="wv_sb")
        nc.sync.dma_start(t, wg_d["wv"])
        wg["wv"] = t
        t = wres.tile([HD, NH, DIM], f8, name="wp_sb")
        nc.scalar.dma_start(t, wg_d["wp"])
        wg["wp"] = t
        dg1 = []
        for c in range(NH):
            t = wres.tile([HD, 9, 2 * HD], f8, name=f"dg1_{c}")
            nc.sync.dma_start(t, dg1_d[:, c])
            dg1.append(t)
        dg2p = []
        for c in range(NH):
            t = wres.tile([HD, 4, 2 * HD], f8, name=f"dg2p_{c}")
            nc.sync.dma_start(t, dg2p_d[:, c])
            dg2p.append(t)
        dg2s = []
        for c in range(NH):
            t = wres.tile([HD, HD], f8, name=f"dg2s_{c}")
            nc.sync.dma_start(t, dg2s_d[:, c])
            dg2s.append(t)
        pv = [[cpack[:, br, c, 0:9] for c in range(NH)] for br in range(2)]
        seb2 = [[cpack[:, br, c, 41:42] for c in range(NH)] for br in range(2)]
        bias1 = [[cpack[:, br, c, 42:43] for c in range(NH)] for br in range(2)]
        dwcb = [cpack[:, 0, c, 43:44] for c in range(NH)]

        # ---------- persistent SBUF ----------
        qk = [big.tile([HD, 2 * BUFN], f8, name=f"qk{c}") for c in range(NH)]
        m_t = [big.tile([HD, BUFN], f8, name=f"m{c}") for c in range(NH)]
        v_t = big.tile([HD, NH, HW], f8, name="v")
        o2_t = big.tile([HD, NH, HW], f8, name="o2")
        hvz = big.tile([HD, 2, NH], f32, name="hvz")
        nc.gpsimd.memset(hvz, 0.0)
        # dummy sigmoid: pulls the sigmoid act-table load (1.3us) into the
        # startup DMA wait instead of the first sample's SE chain
        warm = big.tile([HD, 1], f32, name="warm")
        nc.scalar.activation(warm, hvz[:, 0, 0:1], AF.Sigmoid, bias=0.0)

        # zero only the pad cells that valid conv outputs read (GPSIMD:
        # off the DVE critical path at startup)
        def zero_pads(plane_base, tbuf):
            for start, stride, count in (
                    (plane_base + HEAD, 1, 58),
                    (plane_base + HEAD + 57 * WP, 1, 58),
                    (plane_base + HEAD, WP, 58),
                    (plane_base + HEAD + 57, WP, 58)):
                u = tbuf[:, start:start + 1 + stride * (count - 1)]
                a = u.ap
                a.pop(1)
                a.insert(1, (stride, count))
                nc.gpsimd.memset(u, 0.0)

        for c in range(NH):
            zero_pads(0, qk[c])
            zero_pads(BUFN, qk[c])
            zero_pads(0, m_t[c])

        def interior(tbuf, plane_base, t):
            """[128, 8, 56] view of padded rows 1+8t..8+8t, cols 1..56."""
            base = plane_base + HEAD + (1 + TH * t) * WP + 1
            u = tbuf[:, base:base + TH * WP]
            return u.rearrange("p (r c) -> p r c", c=WP)[:, :, 0:W]

        def issue_x8(b):
            if b == 0:
                return x8_first
            x8 = x8pool.tile([HD, NH, HWS], f8, tag="x8", name=f"x8_b{b}")
            x8_src = x8_d[b].rearrange("(kc p) h w -> p kc (h w)", p=HD)
            for kc in range(0, NH, 2):
                nc.sync.dma_start(x8[:, kc:kc + 2, 0:HW],
                                  x8_src[:, kc:kc + 2, :])
            return x8

        def emit_sample(b, x8, x8_next):
            sfx = f"_b{b}"
            # ------- x strip/corner stats into the x8 tile's tail -------
            # col HW+j, j: 0 row0, 1 row55, 2 col0, 3 col55,
            #             4 c(0,0), 5 c(0,55), 6 c(55,0), 7 c(55,55)
            for j, (lo, step) in enumerate(
                    ((0, 1), (55 * W, 1), (0, W), (55, W))):
                src = x8[:, :, lo:lo + 1 + step * (W - 1)]
                a = src.ap
                a.pop(2)
                a.insert(2, (step, W))
                with nc.allow_low_precision(reason="fp8 strip sums feed "
                                            "small SE border corrections"):
                    nc.vector.tensor_reduce(x8[:, :, HW + j:HW + j + 1],
                                            src, AX.X, AL.add)
            for j, lo in ((4, 0), (6, 55 * W)):
                src = x8[:, :, lo:lo + 56]
                a = src.ap
                a.pop(2)
                a.insert(2, (55, 2))
                nc.vector.tensor_scalar(x8[:, :, HW + j:HW + j + 2],
                                        src, 1.0, None, AL.mult)

            xfs = []
            for ocp in range(NH):
                xf = xfpool.tile([HD, HW], bf16, tag="xf",
                                 name=f"xf{ocp}{sfx}")
                nc.sync.dma_start(
                    xf.rearrange("p (h w) -> p h w", w=W),
                    xf_d[b, ocp * HD:(ocp + 1) * HD])
                xfs.append(xf)

            # ---- qk GEMM; t=6 runs FD=456, its last 8 psum columns are
            # the W*(x strips) = q~/k~ border sums ----
            stats = [None] * NH
            corr4 = small.tile([HD, 2, NH], f32, tag="corr4",
                               name=f"corr4{sfx}")
            for oc in range(NH):
                stats[oc] = statp.tile([HD, 2, 9], f32, tag="stat",
                                       name=f"stat{oc}{sfx}")
                for br in range(2):
                    wt = wg["wq"] if br == 0 else wg["wk"]
                    st = statp.tile([HD, NT], f32, tag="st",
                                    name=f"st{br}_{oc}{sfx}")
                    for t in range(NT):
                        fd = TN + NST if t == NT - 1 else TN
                        ps = mmp.tile([HD, fd], f32, tag="mm",
                                      name=f"g{br}_{oc}_{t}{sfx}")
                        for kp in range(2):
                            nc.tensor.matmul(
                                ps,
                                wt[:, 2 * kp:2 * kp + 2,
                                   oc * HD:(oc + 1) * HD],
                                x8[:, 2 * kp:2 * kp + 2,
                                   t * TN:t * TN + fd],
                                start=(kp == 0), stop=(kp == 1),
                                perf_mode=DR)
                        if t % 2 == 0 and t != NT - 1:
                            nc.scalar.activation(
                                interior(qk[oc], br * BUFN, t),
                                ps[:, 0:TN].rearrange(
                                    "p (r c) -> p r c", c=W),
                                AF.Identity, bias=0.0, scale=1.0 / SCALE_W,
                                accum_out=st[:, t:t + 1])
                        else:
                            nc.vector.tensor_scalar(
                                interior(qk[oc], br * BUFN, t),
                                ps[:, 0:TN].rearrange(
                                    "p (r c) -> p r c", c=W),
                                1.0 / SCALE_W, 0.0, AL.mult, AL.add,
                                accum_out=st[:, t:t + 1])
                        if t == NT - 1:
                            nc.vector.tensor_scalar(
                                stats[oc][:, br, 1:9], ps[:, TN:TN + NST],
                                1.0 / SCALE_W, None, AL.mult)
                    nc.vector.tensor_reduce(stats[oc][:, br, 0:1],
                                            st[:, 0:NT], AX.X, AL.add)
                    tmp9 = small.tile([HD, 9], f32, tag="tmp9", name=None)
                    nc.gpsimd.tensor_tensor(tmp9, stats[oc][:, br, :],
                                            pv[br][oc], AL.mult)
                    nc.vector.tensor_reduce(corr4[:, br, oc:oc + 1],
                                            tmp9, AX.X, AL.add)

            # v GEMM for oc 0-1: PE filler while corr/SE chain completes
            def v_gemm(oc):
                for t in range(NT):
                    ps = mmp.tile([HD, TN], f32, tag="mm",
                                  name=f"v{oc}_{t}{sfx}")
                    for kp in range(2):
                        nc.tensor.matmul(
                            ps,
                            wg["wv"][:, 2 * kp:2 * kp + 2,
                                     oc * HD:(oc + 1) * HD],
                            x8[:, 2 * kp:2 * kp + 2, t * TN:(t + 1) * TN],
                            start=(kp == 0), stop=(kp == 1), perf_mode=DR)
                    nc.vector.tensor_scalar(
                        v_t[:, oc, t * TN:(t + 1) * TN], ps,
                        1.0 / SCALE_W, None, AL.mult)

            v_gemm(0)
            v_gemm(1)
            v_gemm(2)

            s_sb = [[None] * NH for _ in range(2)]
            for br in range(2):
                ps1 = sep.tile([HD, NH], f32, tag="se",
                               name=f"se1_{br}{sfx}")
                nc.tensor.matmul(ps1, sew1b[:, br, :], corr4[:, br, :],
                                 start=True, stop=True)
                # relu only the diagonal quadrants into the pre-zeroed
                # hvz so se2 can contract all 128 partitions at once
                for f in range(NH):
                    nc.scalar.activation(
                        hvz[f * HD4:(f + 1) * HD4, br, f:f + 1],
                        ps1[f * HD4:(f + 1) * HD4, f:f + 1],
                        AF.Relu, bias=sb1[f * HD4:(f + 1) * HD4,
                                          br:br + 1])
            for br in range(2):
                ps2 = sep.tile([HD, NH], f32, tag="se",
                               name=f"se2_{br}{sfx}")
                nc.tensor.matmul(ps2, sew2b[:, br, :], hvz[:, br, :],
                                 start=True, stop=True)
                for oc in range(NH):
                    sv = small.tile([HD, 1], f32, tag="sv",
                                    name=f"s{br}_{oc}{sfx}")
                    nc.scalar.activation(sv, ps2[:, oc:oc + 1], AF.Sigmoid,
                                         bias=seb2[br][oc])
                    s_sb[br][oc] = sv

            # ------- v GEMM oc 3 (filler while dg1s scaling runs) -------
            v_gemm(3)

            # prefetch next sample's x8 ahead of this sample's out stores
            if x8_next is not None:
                x8n = issue_x8(b + 1)
            else:
                x8n = None

            # ------- scale conv1 stationaries (ACT) + bias_m -------
            dg1s = [None] * NH
            bias_m = [None] * NH
            for oc in range(NH):
                gt = dg1s_p.tile([HD, 9, 2 * HD], f8, tag=f"dg1s{oc}",
                                 name=f"dg1s{oc}{sfx}")
                nc.scalar.activation(gt[:, 0:3, :], dg1[oc][:, 0:3, :],
                                     AF.Identity, bias=0.0,
                                     scale=s_sb[0][oc])
                nc.vector.tensor_scalar(gt[:, 3:6, :], dg1[oc][:, 3:6, :],
                                        s_sb[1][oc], None, AL.mult)
                gc = gt[:, 6:9, :].rearrange("p j (two f) -> p j two f",
                                             two=2)
                dc = dg1[oc][:, 6:9, :].rearrange("p j (two f) -> p j two f",
                                                  two=2)
                nc.scalar.activation(gc[:, :, 0, :], dc[:, :, 0, :],
                                     AF.Identity, bias=0.0,
                                     scale=s_sb[0][oc])
                nc.vector.tensor_scalar(gc[:, :, 1, :], dc[:, :, 1, :],
                                        s_sb[1][oc], None, AL.mult)
                dg1s[oc] = gt
                bm = small.tile([HD, 1], f32, tag="bm", name=f"bm{oc}{sfx}")
                tmp = small.tile([HD, 1], f32, tag="bmt", name=None)
                nc.vector.tensor_scalar(tmp, bias1[0][oc], s_sb[0][oc],
                                        None, AL.mult)
                nc.vector.scalar_tensor_tensor(bm, bias1[1][oc], s_sb[1][oc],
                                               tmp, AL.mult, AL.add)
                bias_m[oc] = bm

            # ---------------- conv1 (fused q+k -> m) ----------------
            for oc in range(NH):
                qplane = qk[oc]
                for t in range(NT):
                    y0 = 1 + TH * t
                    ps = mmp.tile([HD, FDC], f32, tag="mm",
                                  name=f"c1_{oc}_{t}{sfx}")
                    mms = []
                    for i in range(9):
                        grp, dx = divmod(i, 3)
                        dx -= 1
                        if grp == 0:    # q vertical pairs
                            mv = pairify(qplane,
                                         HEAD + (y0 - 1) * WP + dx,
                                         FDC, 2 * WP)
                        elif grp == 1:  # k vertical pairs
                            mv = pairify(qplane,
                                         BUFN + HEAD + (y0 - 1) * WP + dx,
                                         FDC, 2 * WP)
                        else:           # cross q/k middle row
                            mv = pairify(qplane, HEAD + y0 * WP + dx,
                                         FDC, BUFN)
                        st = dg1s[oc][:, i, :]
                        st = st.rearrange("p (two f) -> p two f", two=2)
                        mms.append((st, mv))
                    for i, (st, mv) in enumerate(mms):
                        nc.tensor.matmul(ps, st, mv, start=(i == 0),
                                         stop=(i == 8), perf_mode=DR)
                    nc.scalar.activation(
                        interior(m_t[oc], 0, t),
                        ps.rearrange("p (r c) -> p r c", c=WP)[:, :, 1:1 + W],
                        AF.Identity, bias=bias_m[oc], scale=1.0 / SCALE_W)

            # ------- conv2(t) + proj(t-1), one-tile lag -------
            stgs = [None] * NH
            for t in range(NT + 1):
                if t < NT:
                    for oc in range(NH):
                        y0 = 1 + TH * t
                        ps = mmp.tile([HD, FDC], f32, tag="mm",
                                      name=f"c2_{oc}_{t}{sfx}")
                        for i in range(3):
                            dx = i - 1
                            st = dg2p[oc][:, i, :]
                            st = st.rearrange("p (two f) -> p two f", two=2)
                            mv = pairify(m_t[oc], HEAD + (y0 - 1) * WP + dx,
                                         FDC, 2 * WP)
                            nc.tensor.matmul(ps, st, mv, start=(i == 0),
                                             stop=False, perf_mode=DR)
                        st = dg2p[oc][:, 3, :]
                        st = st.rearrange("p (two f) -> p two f", two=2)
                        mv = pairify(m_t[oc], HEAD + y0 * WP - 1, FDC, 2)
                        nc.tensor.matmul(ps, st, mv, start=False, stop=False,
                                         perf_mode=DR)
                        nc.tensor.matmul(
                            ps, dg2s[oc],
                            m_t[oc][:, HEAD + y0 * WP:HEAD + y0 * WP + FDC],
                            start=False, stop=True)
                        c2t = c2pool.tile([HD, TN], f8, tag="c2t",
                                          name=f"c2t{oc}_{t}{sfx}")
                        nc.scalar.activation(
                            c2t.rearrange("p (r c) -> p r c", c=W),
                            ps.rearrange("p (r c) -> p r c",
                                         c=WP)[:, :, 1:1 + W],
                            AF.Identity, bias=dwcb[oc],
                            scale=CSCALE / SCALE_W)
                        eng = nc.vector if oc == 0 else nc.gpsimd
                        eng.tensor_tensor(
                            o2_t[:, oc, t * TN:(t + 1) * TN], c2t,
                            v_t[:, oc, t * TN:(t + 1) * TN], AL.mult)
                if t >= 1:
                    tp = t - 1
                    for ocp in range(NH):
                        xf = xfs[ocp]
                        ps = mmp.tile([HD, TN], f32, tag="mm",
                                      name=f"p{ocp}_{tp}{sfx}")
                        for kp in range(2):
                            nc.tensor.matmul(
                                ps,
                                wg["wp"][:, 2 * kp:2 * kp + 2,
                                         ocp * HD:(ocp + 1) * HD],
                                o2_t[:, 2 * kp:2 * kp + 2,
                                     tp * TN:(tp + 1) * TN],
                                start=(kp == 0), stop=(kp == 1),
                                perf_mode=DR)
                        if tp % 2 == 0:
                            stg = stage.tile([HD, 2, TN], bf16, tag="stg",
                                             name=f"stg{ocp}_{tp}{sfx}")
                            stgs[ocp] = stg
                        else:
                            stg = stgs[ocp]
                        nc.vector.scalar_tensor_tensor(
                            stg[:, tp % 2, :], ps, 1.0 / (SCALE_W * CSCALE),
                            xf[:, tp * TN:(tp + 1) * TN], AL.mult, AL.add)
                        if tp % 2 == 1 or tp == NT - 1:
                            t0o = tp - (tp % 2)
                            deng = nc.scalar if (tp >= NT - 2
                                                 and ocp % 2 == 1) else nc.sync
                            deng.dma_start(
                                out_d[b, ocp * HD:(ocp + 1) * HD,
                                      TH * t0o:TH * (tp + 1), :],
                                stg[:, 0:tp - t0o + 1, :].rearrange(
                                    "p u (r c) -> p (u r) c", c=W))
            return x8n

        x8 = issue_x8(0)
        for b in range(BL):
            x8 = emit_sample(b, x8, b + 1 < BL or None)

    nc.compile()
    return nc


# ---------------------------------------------------------------------------
# host-side weight prep
# ---------------------------------------------------------------------------

def prep_weights(inputs, cfg):
    import ml_dtypes
    f8 = ml_dtypes.float8_e4m3
    f32 = np.float32

    qkv_w = np.asarray(inputs['qkv_w'], f32)
    proj_w = np.asarray(inputs['proj_w'], f32)

    def gemm_tile(wmat):
        # [HD p, NH kc, DIM oc*128+col] = SCALE_W * W[ocol, kc*128+p]
        arr = (SCALE_W * wmat).reshape(DIM, NH, HD).transpose(2, 1, 0)
        return np.ascontiguousarray(arr).astype(f8)

    wq = gemm_tile(qkv_w[0:DIM])
    wk = gemm_tile(qkv_w[DIM:2 * DIM])
    wv = gemm_tile(qkv_w[2 * DIM:3 * DIM])
    wp = gemm_tile(proj_w)

    sq = np.asarray(inputs['sq_w'], f32).reshape(DIM, 3, 3) * SCALE_W
    sk = np.asarray(inputs['sk_w'], f32).reshape(DIM, 3, 3) * SCALE_W
    dw = np.asarray(inputs['dwc_w'], f32).reshape(DIM, 3, 3) * SCALE_W

    idx = np.arange(HD)

    def diag(vals):
        d = np.zeros((HD, HD), f32)
        d[idx, idx] = vals
        return d

    # conv1 pair stationaries: [NH, 9, HD, 2*HD]
    dg1 = np.zeros((NH, 9, HD, 2 * HD), f32)
    for c in range(NH):
        q = sq[c * HD:(c + 1) * HD]
        k = sk[c * HD:(c + 1) * HD]
        for i in range(3):      # q vertical pairs, dx = i-1
            dg1[c, i, :, 0:HD] = diag(q[:, 0, i])
            dg1[c, i, :, HD:] = diag(q[:, 2, i])
        for i in range(3):      # k vertical pairs
            dg1[c, 3 + i, :, 0:HD] = diag(k[:, 0, i])
            dg1[c, 3 + i, :, HD:] = diag(k[:, 2, i])
        for i in range(3):      # cross middle row
            dg1[c, 6 + i, :, 0:HD] = diag(q[:, 1, i])
            dg1[c, 6 + i, :, HD:] = diag(k[:, 1, i])

    dg2p = np.zeros((NH, 4, HD, 2 * HD), f32)
    dg2s = np.zeros((NH, HD, HD), f32)
    for c in range(NH):
        d = dw[c * HD:(c + 1) * HD]
        for i in range(3):      # vertical pairs
            dg2p[c, i, :, 0:HD] = diag(d[:, 0, i])
            dg2p[c, i, :, HD:] = diag(d[:, 2, i])
        dg2p[c, 3, :, 0:HD] = diag(d[:, 1, 0])   # (0,-1)
        dg2p[c, 3, :, HD:] = diag(d[:, 1, 2])    # (0,+1)
        dg2s[c] = diag(d[:, 1, 1])

    # pooled-correction vectors (natural scale, /npix folded in):
    # pooled = A*S + B1*R0 + B2*R1 + B3*C0 + B4*C1 + c00*q00 + ...
    npix = float(H * W)
    pvv = np.zeros((2, DIM, 9), f32)
    for br, wc in enumerate((sq / SCALE_W, sk / SCALE_W)):
        pvv[br, :, 0] = wc.sum(axis=(1, 2))
        pvv[br, :, 1] = -wc[:, 2, :].sum(axis=1)   # row 0 strip (dy=+1 taps)
        pvv[br, :, 2] = -wc[:, 0, :].sum(axis=1)   # row 55 strip (dy=-1)
        pvv[br, :, 3] = -wc[:, :, 2].sum(axis=1)   # col 0 strip (dx=+1)
        pvv[br, :, 4] = -wc[:, :, 0].sum(axis=1)   # col 55 strip (dx=-1)
        pvv[br, :, 5] = wc[:, 2, 2]                # q[0,0]
        pvv[br, :, 6] = wc[:, 2, 0]                # q[0,55]
        pvv[br, :, 7] = wc[:, 0, 2]                # q[55,0]
        pvv[br, :, 8] = wc[:, 0, 0]                # q[55,55]
    pvv /= npix

    sew1 = np.stack([
        np.asarray(inputs['cq_w1'], f32).transpose(0, 2, 1),
        np.asarray(inputs['ck_w1'], f32).transpose(0, 2, 1)])  # [2,NH,HD,HD4]
    seb1 = np.stack([np.asarray(inputs['cq_b1'], f32),
                     np.asarray(inputs['ck_b1'], f32)])        # [2,NH,HD4]
    sew2 = np.stack([
        np.asarray(inputs['cq_w2'], f32).transpose(0, 2, 1),
        np.asarray(inputs['ck_w2'], f32).transpose(0, 2, 1)])  # [2,NH,HD4,HD]
    seb2 = np.stack([np.asarray(inputs['cq_b2'], f32),
                     np.asarray(inputs['ck_b2'], f32)])        # [2,NH,HD]
    b1 = np.stack([np.asarray(inputs['sq_b'], f32),
                   np.asarray(inputs['sk_b'], f32)])           # [2,DIM]
    dwcb = CSCALE * np.asarray(inputs['dwc_b'], f32)           # [DIM]

    cpack = np.zeros((2, NH, HD, 44), f32)
    for br in range(2):
        for c in range(NH):
            sl = slice(c * HD, (c + 1) * HD)
            cpack[br, c, :, 0:9] = pvv[br, sl]
            cpack[br, c, :, 41] = seb2[br, c]
            cpack[br, c, :, 38] = np.exp(-seb2[br, c])
            cpack[br, c, :, 42] = b1[br, sl]
    for c in range(NH):
        cpack[0, c, :, 43] = dwcb[c * HD:(c + 1) * HD]
        cpack[0, c, :, 40] = (CSCALE / SCALE_W) * dw[c * HD:(c + 1) * HD, 1, 1]

    # batched se1 stationary [HD, 2, NH*HD4] and relu bias [HD, 2]
    sew1b = np.zeros((HD, 2, NH * HD4), f32)
    for br in range(2):
        sew1b[:, br, :] = np.concatenate(
            [sew1[br, f] for f in range(NH)], axis=1)
    sb1 = np.ascontiguousarray(seb1.reshape(2, HD).T)

    return dict(
        wq=wq, wk=wk, wv=wv, wp=wp,
        dg1=np.ascontiguousarray(
            dg1.transpose(2, 0, 1, 3)).astype(f8),
        dg2p=np.ascontiguousarray(
            dg2p.transpose(2, 0, 1, 3)).astype(f8),
        dg2s=np.ascontiguousarray(dg2s.transpose(1, 0, 2)).astype(f8),
        cpack=np.ascontiguousarray(cpack.transpose(2, 0, 1, 3)),
        sew2b=np.ascontiguousarray(
            sew2.reshape(2, NH * HD4, HD).transpose(1, 0, 2)),
        sew1b=sew1b, sb1=sb1,
    )


_CACHE = {}


def _get_compiled(cfg_key, cfg):
    if cfg_key not in _CACHE:
        _CACHE[cfg_key] = build_nc(cfg)
    return _CACHE[cfg_key]


def make_in_maps(inputs, cfg):
    import ml_dtypes
    w = prep_weights(inputs, cfg)
    x32 = np.asarray(inputs['x'], np.float32)
    x8 = np.clip(x32, -240, 240).astype(ml_dtypes.float8_e4m3)
    projb = np.asarray(inputs['proj_b'], np.float32)
    xf = (x32 + projb[None, :, None, None]).astype(ml_dtypes.bfloat16)
    BL = cfg['b_local']
    in_maps = []
    for core in range(N_CORES):
        mm = dict(w)
        mm['x8'] = np.ascontiguousarray(x8[core * BL:(core + 1) * BL])
        mm['xf'] = np.ascontiguousarray(xf[core * BL:(core + 1) * BL])
        in_maps.append(mm)
    return in_maps


def kernel(**inputs):
    from concourse import bass_utils
    cfg = default_cfg()
    nc = _get_compiled('main', cfg)
    in_maps = make_in_maps(inputs, cfg)
    res = bass_utils.run_bass_kernel_spmd(nc, in_maps,
                                          core_ids=list(range(N_CORES)))
    bl = cfg['b_local']
    out = np.empty((B, DIM, H_FULL, W), np.float32)
    for core in range(N_CORES):
        out[core * bl:(core + 1) * bl] = np.asarray(
            res.results[core]['out'], np.float32)
    return out
